# revision 33
# baseline (speedup 1.0000x reference)
"""NonLocalBlock (B=4, C=64, Ci=32, H=W=64) on 8 TRN2 NeuronCores.

Sharding: data-parallel over batch (4 pairs of cores); within each pair
the query dimension n of the NxN score matrix is split in half.
Softmax runs over n (dim=1), so each core computes partial softmax
denominators S[m] over its n-half; tiny pairwise AllReduces ([128 x g]
f32) produce the full denominators. Everything else is local: each
core produces z[:, n_half] and the host concatenates.

Per core (b = core//2, h = core%2):
  theta2 = [theta|theta] stacked x2                 [64, 2048] bf16
  phi2   = phi, even m-tiles on partitions 0:32,    [64, 2048] bf16
           odd on 32:64 (col-tiled projection)
  fT     = phi^T @ theta, 2x row-tiled pairs        [128, 1024] PSUM f32
  expT   : A-tiles exp on ACT (accum_out -> S),     bf16 SBUF
           D-tiles Schraudolph exp on DVE
           (int16 bf16-bit trick) + DVE reduce
  S      = AllReduce_pair of per-tile row sums
  wgT    = ref_aug^T @ (w_w@g_w | w_w@g_b)^T        [128, 64] per m-tile
  wgT'   = wgT * (1/S)  (softmax scale + final 1x1 conv folded into g)
  z      = sum_mt wgT'^T @ expT  (col-tiled PSUM accum, [128 x 1024]:
           partitions 0:64 = n-cols 0:1024, 64:128 = n-cols 1024:2048)
  out    = supp[:, nh] + z + w_b  (DVE, DMA'd as repacked [128, 1024])
"""

import numpy as np

B, C, CI, H, W = 4, 64, 32, 64, 64
N = H * W            # 4096
NLOC = N // 2        # 2048 n-columns per core
NCORES = 8
MTP = 128            # m-tile partition size
NMT = N // MTP       # 32 m-tiles
GROUP_SIZES = [8, 8, 8, 4, 4]
# Schraudolph-on-DVE tiles (never last in group; odd pair members)
D_TILES = frozenset([1, 3, 5, 9, 11, 17, 19, 21, 25])
CK = 512             # matmul moving-dim chunk
SCH_A = 128.0 / np.log(2.0)        # bf16-bit Schraudolph: i16 = A*x + B
SCH_B = 128.0 * (127.0 - 0.045) + 0.5

REPLICA_GROUPS = [[0, 1], [2, 3], [4, 5], [6, 7]]

_cache = {}


def _build():
    import concourse.bacc as bacc
    import concourse.tile as tile
    from concourse import mybir

    f32 = mybir.dt.float32
    bf16 = mybir.dt.bfloat16
    i16 = mybir.dt.int16
    AF = mybir.ActivationFunctionType
    ALU = mybir.AluOpType

    nc = bacc.Bacc(None, target_bir_lowering=False, debug=False)

    supp_rp = nc.dram_tensor("supp_rp", [MTP, NLOC // 2], f32, kind="ExternalInput")
    supp_b = nc.dram_tensor("supp_b", [C, NLOC], bf16, kind="ExternalInput")
    ref_aug = nc.dram_tensor("ref_aug", [C + 1, N], bf16, kind="ExternalInput")
    # packed weights (zero-padded to 128 rows): cols 0:32 theta_wT,
    # 32:64 phi_wT, 96:160 wg_aug
    wcomb_d = nc.dram_tensor("wcomb", [MTP, 160], bf16, kind="ExternalInput")
    # col 0 = theta_b, col 1 = phi_b (rows 0:32)
    tbpb_d = nc.dram_tensor("tbpb", [C, 2], f32, kind="ExternalInput")
    w_bc2 = nc.dram_tensor("w_bc2", [MTP, 1], f32, kind="ExternalInput")
    out = nc.dram_tensor("out", [MTP, NLOC // 2], f32, kind="ExternalOutput")

    NG = len(GROUP_SIZES)
    assert sum(GROUP_SIZES) == NMT
    group_of = []
    for g, gs in enumerate(GROUP_SIZES):
        group_of += [g] * gs
    group_start = [sum(GROUP_SIZES[:g]) for g in range(NG)]

    with tile.TileContext(nc) as tc:
        from contextlib import ExitStack

        with ExitStack() as ctx:
            sing = ctx.enter_context(tc.tile_pool(name="sing", bufs=1))
            spool = ctx.enter_context(tc.tile_pool(name="spool", bufs=NG))
            epool = ctx.enter_context(tc.tile_pool(name="expT", bufs=NMT))
            dpool = ctx.enter_context(
                tc.tile_pool(name="dram", bufs=NG, space="DRAM")
            )
            outp = ctx.enter_context(tc.tile_pool(name="outp", bufs=2))

            # ---------------- loads ----------------
            # each engine queue is one serial DMA stream (~90 GB/s), so
            # split the big loads across sync/scalar/gpsimd
            wcomb = sing.tile([MTP, 160], bf16, tag="wcomb")
            nc.gpsimd.dma_start(out=wcomb, in_=wcomb_d[:, :])
            tbpb = sing.tile([C, 2], f32, tag="tbpb")
            nc.gpsimd.dma_start(out=tbpb, in_=tbpb_d[:, :])
            wb = sing.tile([MTP, 1], f32, tag="wb")
            nc.gpsimd.dma_start(out=wb, in_=w_bc2[:, :])
            supp_bf = sing.tile([C, NLOC], bf16, tag="suppbf")
            nc.gpsimd.dma_start(out=supp_bf, in_=supp_b[:, :])
            refa = sing.tile([MTP, N], bf16, tag="refa")
            # zero the pad rows first (quad-aligned spans, idle DVE
            # queue), then DMA the real 65 rows on two queues
            nc.vector.memset(refa[2 * CI : 3 * CI, :], 0.0)
            nc.vector.memset(refa[3 * CI : MTP, :], 0.0)
            nc.sync.dma_start(out=refa[0 : C + 1, 0:NLOC], in_=ref_aug[:, 0:NLOC])
            nc.scalar.dma_start(out=refa[0 : C + 1, NLOC:N], in_=ref_aug[:, NLOC:N])
            supp_t = sing.tile([MTP, NLOC // 2], f32, tag="supp")
            nc.scalar.dma_start(out=supp_t, in_=supp_rp[:, :])

            tw = wcomb[0:C, 0:CI]
            pw = wcomb[0:C, CI : 2 * CI]
            wga = wcomb[:, C + CI : C + CI + C]
            tb = tbpb[0:CI, 0:1]
            pb = tbpb[0:CI, 1:2]

            # All stream matmuls are padded to K=128 (zero rows): the
            # matmul cost is N-paced so padding is free, and full-array
            # activity keeps the HAM clock-gate at 2.4 GHz; partial-array
            # (K=32/K=65) streams get throttled to 1.2 GHz.
            theta_pad = sing.tile([MTP, NLOC], bf16, tag="thetap")
            phi_pad = sing.tile([MTP, N], bf16, tag="phip")
            nc.vector.memset(theta_pad[CI : 2 * CI, :], 0.0)
            nc.vector.memset(theta_pad[2 * CI : 3 * CI, :], 0.0)
            nc.vector.memset(theta_pad[3 * CI : MTP, :], 0.0)
            nc.gpsimd.memset(phi_pad[CI : 2 * CI, :], 0.0)
            nc.gpsimd.memset(phi_pad[2 * CI : 3 * CI, :], 0.0)
            nc.gpsimd.memset(phi_pad[3 * CI : MTP, :], 0.0)
            wgt_raw = sing.tile([MTP, NMT * C], f32, tag="wgtraw")
            wgt_b16 = sing.tile([MTP, NMT * C], bf16, tag="wgtb16")
            scr_v = sing.tile([MTP, NLOC], bf16, tag="scrv")

            # -------- early sync barrier --------
            # The 8 cores start with multi-us NEFF-launch skew; every
            # AllReduce rendezvous re-exposes it. A dummy 1-element
            # AllReduce first thing absorbs the skew during the DMA-load
            # phase, so the real denominator exchanges run ~6us.
            bar_in = dpool.tile([1, 1], f32, tag="barin")
            bar_out = dpool.tile([1, 1], f32, tag="barout")
            bar_s = sing.tile([1, 1], f32, tag="bars")
            nc.gpsimd.memset(bar_s[:, :], 1.0)
            nc.gpsimd.dma_start(out=bar_in, in_=bar_s)
            nc.gpsimd.collective_compute(
                "AllReduce",
                ALU.add,
                replica_groups=REPLICA_GROUPS,
                ins=[bar_in.opt()],
                outs=[bar_out.opt()],
            )

            # -------- PE warm-up burst --------
            # ~5us of dense full-array (K=128) matmuls so the HAM
            # clock-gate releases (1.2 -> 2.4 GHz): partial-array work
            # (K=32 fT, K=65 wgT) does not register enough activity.
            # Sources a memset scratch (no DMA dependency -> starts right
            # after the preamble, overlapping the input loads) and rotates
            # 4 PSUM banks so WAW never stalls the PE.
            warm_src = sing.tile([MTP, CK], bf16, tag="warmsrc")
            nc.vector.memset(warm_src[:, :], 0.0)
            warm_ctx = ExitStack()
            warmp = warm_ctx.enter_context(
                tc.tile_pool(name="warmp", bufs=4, space="PSUM")
            )
            for k in range(12):
                wps = warmp.tile([MTP, CK], f32, tag="warm", name=f"warm{k}")
                nc.tensor.matmul(
                    wps,
                    lhsT=warm_src[:, 0:MTP],
                    rhs=warm_src[:, :],
                    start=True,
                    stop=True,
                )
            warm_ctx.close()

            # -------- projections --------
            # Bias adds alternate ACT/DVE so neither serializes the phase.
            psA_ctx = ExitStack()
            psA = psA_ctx.enter_context(
                tc.tile_pool(name="psA", bufs=4, space="PSUM")
            )
            proj_alt = [0]

            def emit_add(dst, ps, bias):
                # first chain (phi0/th0/th1 gate the first fT) alternates
                # ACT/DVE; everything later goes to DVE (ACT is the
                # stream bottleneck, DVE has early slack)
                if proj_alt[0] in (0, 2):
                    nc.scalar.activation(out=dst, in_=ps, func=AF.Identity, bias=bias)
                else:
                    nc.vector.tensor_scalar_add(dst, ps, bias)
                proj_alt[0] += 1

            def emit_phi(j):
                ps = psA.tile([CI, CK], f32, tag="projps", name=f"phi_ps{j}")
                nc.tensor.matmul(
                    ps,
                    lhsT=pw,
                    rhs=refa[0:C, j * CK : (j + 1) * CK],
                    start=True,
                    stop=True,
                )
                emit_add(phi_pad[0:CI, j * CK : (j + 1) * CK], ps, pb)

            def emit_theta(j):
                ps = psA.tile([CI, CK], f32, tag="projps", name=f"th_ps{j}")
                nc.tensor.matmul(
                    ps,
                    lhsT=tw,
                    rhs=supp_bf[:, j * CK : (j + 1) * CK],
                    start=True,
                    stop=True,
                )
                emit_add(theta_pad[0:CI, j * CK : (j + 1) * CK], ps, tb)

            # phi chunk 0 + theta first: unblocks fT of tiles 0-3
            emit_phi(0)
            for j in range(NLOC // CK):
                emit_theta(j)
            for j in range(1, N // CK):
                emit_phi(j)
            psA_ctx.close()

            ftp = ctx.enter_context(tc.tile_pool(name="ftp", bufs=3, space="PSUM"))
            wgt_ctx = ExitStack()
            wgtp = wgt_ctx.enter_context(
                tc.tile_pool(name="wgtp", bufs=2, space="PSUM")
            )

            state = {"z": None}
            wgt_queue = list(range(NMT))
            ets = [None] * NMT
            srecs = [None] * NG

            def emit_wgt(mt):
                ps = wgtp.tile([MTP, C], f32, tag="wgtps")
                nc.tensor.matmul(
                    ps,
                    lhsT=refa[:, mt * MTP : (mt + 1) * MTP],
                    rhs=wga,
                    start=True,
                    stop=True,
                )
                nc.vector.tensor_copy(wgt_raw[:, mt * C : (mt + 1) * C], ps)

            def emit_c(mt):
                g = group_of[mt]
                tl = mt - group_start[g]
                nc.vector.tensor_scalar_mul(
                    wgt_b16[:, mt * C : (mt + 1) * C],
                    wgt_raw[:, mt * C : (mt + 1) * C],
                    srecs[g][:, tl : tl + 1],
                )
                # col-tiled z: partitions 0:64 accumulate n 0:1024,
                # partitions 64:128 accumulate n 1024:2048
                z = state["z"]
                w = wgt_b16[:, mt * C : (mt + 1) * C]
                e = ets[mt]
                for jj in range(2):
                    for ph in range(2):
                        nc.tensor.matmul(
                            z[ph * C : (ph + 1) * C, jj * CK : (jj + 1) * CK],
                            lhsT=w,
                            rhs=e[:, ph * 1024 + jj * CK : ph * 1024 + (jj + 1) * CK],
                            start=(mt == 0),
                            stop=(mt == NMT - 1),
                        )

            # Emission-time model (times relative to first exp, ACT-paced).
            TILE_T = 2.1
            CC_GAP = 2.5
            MARGIN = 2.0
            PE_LAG = 2
            cc_land = [None] * NG
            c_ready = []

            def dribble(mt):
                # wgT matmuls 4/slot on slots 2..9; then z work, <=3/slot
                if wgt_queue:
                    if mt >= 2:
                        for _ in range(4):
                            emit_wgt(wgt_queue.pop(0))
                        if not wgt_queue:
                            wgt_ctx.close()
                            zpp = ctx.enter_context(
                                tc.tile_pool(name="zpp", bufs=1, space="PSUM")
                            )
                            state["z"] = zpp.tile(
                                [MTP, NLOC // 2], f32, tag="z", name="z_ps"
                            )
                    return
                pe_now = (mt - PE_LAG) * TILE_T
                budget = 3
                while budget and c_ready:
                    mt2 = c_ready[0]
                    land = cc_land[group_of[mt2]]
                    if land is not None and pe_now >= land + MARGIN:
                        emit_c(c_ready.pop(0))
                        budget -= 1
                    else:
                        break

            for g, gs in enumerate(GROUP_SIZES):
                s2 = spool.tile([MTP, 2 * gs], f32, tag=f"s2{g}")
                nc.gpsimd.memset(s2[:, :], 0.0)
                d_lag = 0.3
                for tl in range(gs):
                    mt = group_start[g] + tl
                    et = epool.tile([MTP, NLOC], bf16, tag="et", name=f"et{mt}")
                    ets[mt] = et
                    for hh in range(2):
                        ft = ftp.tile([MTP, 2 * CK], f32, tag="ft")
                        for jj in range(2):
                            j = 2 * hh + jj
                            nc.tensor.matmul(
                                ft[:, jj * CK : (jj + 1) * CK],
                                lhsT=phi_pad[:, mt * MTP : (mt + 1) * MTP],
                                rhs=theta_pad[:, j * CK : (j + 1) * CK],
                                start=True,
                                stop=True,
                            )
                        dst = et[:, hh * 2 * CK : (hh + 1) * 2 * CK]
                        if mt in D_TILES:
                            nc.vector.tensor_scalar(
                                out=dst.bitcast(i16),
                                in0=ft[:, :],
                                scalar1=SCH_A,
                                scalar2=SCH_B,
                                op0=ALU.mult,
                                op1=ALU.add,
                            )
                        else:
                            nc.scalar.activation(
                                out=dst,
                                in_=ft[:, :],
                                func=AF.Exp,
                                accum_out=s2[:, 2 * tl + hh : 2 * tl + hh + 1],
                            )
                    if mt in D_TILES:
                        nc.vector.tensor_scalar(
                            out=scr_v[:, :],
                            in0=et[:, :],
                            scalar1=1.0,
                            scalar2=0.0,
                            op0=ALU.mult,
                            op1=ALU.add,
                            accum_out=s2[:, 2 * tl : 2 * tl + 1],
                        )
                        d_lag = 3.0
                    dribble(mt)
                # group complete: exchange softmax denominators (keep the
                # gpsimd queue empty so the CC trigger fires promptly)
                stot = spool.tile([MTP, gs], f32, tag=f"stot{g}")
                nc.gpsimd.tensor_add(
                    stot,
                    s2[:, :].rearrange("p (t q) -> p q t", q=2)[:, 0, :],
                    s2[:, :].rearrange("p (t q) -> p q t", q=2)[:, 1, :],
                )
                cin = dpool.tile([MTP, gs], f32, tag=f"cin{g}")
                cout = dpool.tile([MTP, gs], f32, tag=f"cout{g}")
                nc.sync.dma_start(out=cin, in_=stot)
                nc.gpsimd.collective_compute(
                    "AllReduce",
                    ALU.add,
                    replica_groups=REPLICA_GROUPS,
                    ins=[cin.opt()],
                    outs=[cout.opt()],
                )
                ssum = spool.tile([MTP, gs], f32, tag=f"ssum{g}")
                nc.sync.dma_start(out=ssum, in_=cout)
                srec = spool.tile([MTP, gs], f32, tag=f"srec{g}")
                nc.vector.reciprocal(out=srec, in_=ssum)
                srecs[g] = srec
                launch = (group_start[g] + gs) * TILE_T + d_lag + 1.2
                cc_dur = 7.0
                cc_land[g] = max(
                    launch + cc_dur,
                    16.0 if g == 0 else cc_land[g - 1] + CC_GAP,
                )
                c_ready.extend(range(group_start[g], group_start[g] + gs))

            while c_ready:
                emit_c(c_ready.pop(0))

            # ---------------- epilogue ----------------
            for jj in range(2):
                e2 = outp.tile([MTP, CK], f32, tag="e2")
                nc.vector.scalar_tensor_tensor(
                    out=e2,
                    in0=state["z"][:, jj * CK : (jj + 1) * CK],
                    scalar=wb[:, :],
                    in1=supp_t[:, jj * CK : (jj + 1) * CK],
                    op0=ALU.add,
                    op1=ALU.add,
                )
                nc.sync.dma_start(
                    out=out[:, jj * CK : (jj + 1) * CK], in_=e2
                )

    nc.compile()
    return nc


def _get_nc():
    if "nc" not in _cache:
        _cache["nc"] = _build()
    return _cache["nc"]


def kernel(
    supp_feature,
    ref_feature,
    theta_w,
    theta_b,
    phi_w,
    phi_b,
    g_w,
    g_b,
    w_w,
    w_b,
    _trace=False,
):
    import ml_dtypes

    # run_bass_kernel_spmd imports antenv.axon_hooks when tracing is
    # requested; this container's antenv stub lacks that module, so
    # provide a no-op fallback.
    try:
        import antenv.axon_hooks  # noqa: F401
    except ImportError:
        import sys
        import types

        import antenv

        _mod = types.ModuleType("antenv.axon_hooks")
        _mod._hook = None
        _mod.get_axon_ntff_profile_hook = lambda: _mod._hook
        _mod.set_axon_ntff_profile_hook = lambda h: setattr(_mod, "_hook", h)
        sys.modules["antenv.axon_hooks"] = _mod
        antenv.axon_hooks = _mod

    from concourse.bass_utils import run_bass_kernel_spmd

    bf = ml_dtypes.bfloat16
    supp_feature = np.asarray(supp_feature, dtype=np.float32)
    ref_feature = np.asarray(ref_feature, dtype=np.float32)
    theta_w = np.asarray(theta_w, dtype=np.float32)
    theta_b = np.asarray(theta_b, dtype=np.float32)
    phi_w = np.asarray(phi_w, dtype=np.float32)
    phi_b = np.asarray(phi_b, dtype=np.float32)
    g_w = np.asarray(g_w, dtype=np.float32)
    g_b = np.asarray(g_b, dtype=np.float32)
    w_w = np.asarray(w_w, dtype=np.float32)
    w_b = np.asarray(w_b, dtype=np.float32)

    nc = _get_nc()

    supp2 = supp_feature.reshape(B, C, N)
    ref2 = ref_feature.reshape(B, C, N)
    # Fold the output 1x1 conv into g (weight-only transform):
    #   w_w @ (g_w @ ref + g_b) = (w_w@g_w) @ ref + (w_w@g_b)
    Wg = (w_w @ g_w).astype(np.float32)
    wgb = (w_w @ g_b).astype(np.float32)
    wg_aug = np.concatenate([Wg.T, wgb[None, :]], axis=0)  # [65, 64]
    wcomb = np.zeros((MTP, 160), np.float32)
    wcomb[0:C, 0:CI] = theta_w.T
    wcomb[0:C, CI : 2 * CI] = phi_w.T
    wcomb[0 : C + 1, C + CI : C + CI + C] = wg_aug
    wcomb = np.ascontiguousarray(wcomb.astype(bf))
    tbpb = np.ascontiguousarray(
        np.stack(
            [np.concatenate([theta_b, theta_b]), np.concatenate([phi_b, phi_b])],
            axis=1,
        ).astype(np.float32)
    )
    w_bc2 = np.ascontiguousarray(
        np.concatenate([w_b, w_b]).reshape(MTP, 1).astype(np.float32)
    )

    in_maps = []
    for core in range(NCORES):
        b, h = core // 2, core % 2
        ref_aug = np.ascontiguousarray(
            np.concatenate(
                [ref2[b], np.ones((1, N), np.float32)], axis=0
            ).astype(bf)
        )
        sloc = supp2[b, :, h * NLOC : (h + 1) * NLOC]
        supp_rp = np.ascontiguousarray(
            sloc.reshape(C, 2, NLOC // 2).transpose(1, 0, 2).reshape(MTP, NLOC // 2)
        )
        in_maps.append(
            {
                "supp_rp": supp_rp,
                "supp_b": np.ascontiguousarray(sloc.astype(bf)),
                "ref_aug": ref_aug,
                "wcomb": wcomb,
                "tbpb": tbpb,
                "w_bc2": w_bc2,
            }
        )

    res = run_bass_kernel_spmd(
        nc, in_maps, list(range(NCORES)), trace=_trace
    )
    if _trace:
        _cache["last_exec_time_ns"] = res.exec_time_ns
        _cache["last_results"] = res

    z = np.empty((B, C, N), dtype=np.float32)
    for core in range(NCORES):
        b, h = core // 2, core % 2
        o = res.results[core]["out"]  # [128, 1024]
        z[b, :, h * NLOC : h * NLOC + NLOC // 2] = o[0:C]
        z[b, :, h * NLOC + NLOC // 2 : (h + 1) * NLOC] = o[C:MTP]
    return z.reshape(B, C, H, W)


# revision 44
# speedup vs baseline: 1.1229x; 1.1229x over previous
"""NonLocalBlock (B=4, C=64, Ci=32, H=W=64) on 8 TRN2 NeuronCores.

Sharding: data-parallel over batch (4 pairs of cores); within each pair
the query dimension n of the NxN score matrix is split in half.
Softmax runs over n (dim=1), so each core computes partial softmax
denominators S[m] over its n-half; tiny pairwise AllReduces ([128 x g]
f32) produce the full denominators. Everything else is local: each
core produces z[:, n_half] and the host concatenates.

Per core (b = core//2, h = core%2):
  theta2 = [theta|theta] stacked x2                 [64, 2048] bf16
  phi2   = phi, even m-tiles on partitions 0:32,    [64, 2048] bf16
           odd on 32:64 (col-tiled projection)
  fT     = phi^T @ theta, 2x row-tiled pairs        [128, 1024] PSUM f32
  expT   : A-tiles exp on ACT (accum_out -> S),     bf16 SBUF
           D-tiles Schraudolph exp on DVE
           (int16 bf16-bit trick) + DVE reduce
  S      = AllReduce_pair of per-tile row sums
  wgT    = ref_aug^T @ (w_w@g_w | w_w@g_b)^T        [128, 64] per m-tile
  wgT'   = wgT * (1/S)  (softmax scale + final 1x1 conv folded into g)
  z      = sum_mt wgT'^T @ expT  (col-tiled PSUM accum, [128 x 1024]:
           partitions 0:64 = n-cols 0:1024, 64:128 = n-cols 1024:2048)
  out    = supp[:, nh] + z + w_b  (DVE, DMA'd as repacked [128, 1024])
"""

import numpy as np

B, C, CI, H, W = 4, 64, 32, 64, 64
N = H * W            # 4096
NLOC = N // 2        # 2048 n-columns per core
NCORES = 8
MTP = 128            # m-tile partition size
NMT = N // MTP       # 32 m-tiles
GROUP_SIZES = [8, 8, 8, 4, 4]
# Schraudolph-on-DVE tiles (never last in group; odd pair members)
D_TILES = frozenset([1, 3, 5, 9, 11, 17, 19, 21, 25])
CK = 512             # matmul moving-dim chunk
SCH_A = 128.0 / np.log(2.0)        # bf16-bit Schraudolph: i16 = A*x + B
SCH_B = 128.0 * (127.0 - 0.045) + 0.5

REPLICA_GROUPS = [[0, 1], [2, 3], [4, 5], [6, 7]]

_cache = {}


def _build():
    import concourse.bacc as bacc
    import concourse.tile as tile
    from concourse import mybir

    f32 = mybir.dt.float32
    bf16 = mybir.dt.bfloat16
    i16 = mybir.dt.int16
    AF = mybir.ActivationFunctionType
    ALU = mybir.AluOpType

    nc = bacc.Bacc(None, target_bir_lowering=False, debug=False)

    supp_rp = nc.dram_tensor("supp_rp", [MTP, NLOC // 2], f32, kind="ExternalInput")
    supp_b = nc.dram_tensor("supp_b", [C, NLOC], bf16, kind="ExternalInput")
    ref_aug = nc.dram_tensor("ref_aug", [C + 1, N], bf16, kind="ExternalInput")
    # packed weights (zero-padded to 128 rows): cols 0:32 theta_wT,
    # 32:64 phi_wT, 96:160 wg_aug
    wcomb_d = nc.dram_tensor("wcomb", [MTP, 160], bf16, kind="ExternalInput")
    # col 0 = theta_b, col 1 = phi_b (rows 0:32)
    tbpb_d = nc.dram_tensor("tbpb", [C, 2], f32, kind="ExternalInput")
    w_bc2 = nc.dram_tensor("w_bc2", [MTP, 1], f32, kind="ExternalInput")
    zpad_d = nc.dram_tensor("zpad", [MTP - CI, N], bf16, kind="ExternalInput")
    out = nc.dram_tensor("out", [MTP, NLOC // 2], f32, kind="ExternalOutput")

    NG = len(GROUP_SIZES)
    assert sum(GROUP_SIZES) == NMT
    group_of = []
    for g, gs in enumerate(GROUP_SIZES):
        group_of += [g] * gs
    group_start = [sum(GROUP_SIZES[:g]) for g in range(NG)]

    with tile.TileContext(nc) as tc:
        from contextlib import ExitStack

        with ExitStack() as ctx:
            sing = ctx.enter_context(tc.tile_pool(name="sing", bufs=1))
            spool = ctx.enter_context(tc.tile_pool(name="spool", bufs=NG))
            epool = ctx.enter_context(tc.tile_pool(name="expT", bufs=NMT))
            dpool = ctx.enter_context(
                tc.tile_pool(name="dram", bufs=NG, space="DRAM")
            )
            outp = ctx.enter_context(tc.tile_pool(name="outp", bufs=2))

            # ---------------- loads ----------------
            # each engine queue is one serial DMA stream (~90 GB/s), so
            # split the big loads across sync/scalar/gpsimd
            wcomb = sing.tile([MTP, 160], bf16, tag="wcomb")
            nc.gpsimd.dma_start(out=wcomb, in_=wcomb_d[:, :])
            tbpb = sing.tile([C, 2], f32, tag="tbpb")
            nc.gpsimd.dma_start(out=tbpb, in_=tbpb_d[:, :])
            wb = sing.tile([MTP, 1], f32, tag="wb")
            nc.gpsimd.dma_start(out=wb, in_=w_bc2[:, :])
            supp_bf = sing.tile([C, NLOC], bf16, tag="suppbf")
            nc.gpsimd.dma_start(out=supp_bf, in_=supp_b[:, :])
            refa = sing.tile([C + 1, N], bf16, tag="refa")
            nc.sync.dma_start(out=refa[:, 0:NLOC], in_=ref_aug[:, 0:NLOC])
            nc.scalar.dma_start(out=refa[:, NLOC:N], in_=ref_aug[:, NLOC:N])
            supp_t = sing.tile([MTP, NLOC // 2], f32, tag="supp")
            nc.scalar.dma_start(out=supp_t, in_=supp_rp[:, :])

            tw = wcomb[0:C, 0:CI]
            pw = wcomb[0:C, CI : 2 * CI]
            wga = wcomb[0 : C + 1, C + CI : C + CI + C]
            tb = tbpb[0:CI, 0:1]
            pb = tbpb[0:CI, 1:2]

            # All stream matmuls are padded to K=128 (zero rows): the
            # matmul cost is N-paced so padding is free, and full-array
            # activity keeps the HAM clock-gate at 2.4 GHz; partial-array
            # (K=32/K=65) streams get throttled to 1.2 GHz.
            theta_pad = sing.tile([MTP, NLOC], bf16, tag="thetap")
            phi_pad = sing.tile([MTP, N], bf16, tag="phip")
            # pad rows come from a DRAM zeros tensor (engine memsets of
            # this size cost ~10us; the DMA overlaps the input loads)
            nc.gpsimd.dma_start(
                out=theta_pad[CI:MTP, :], in_=zpad_d[:, 0:NLOC]
            )
            nc.sync.dma_start(out=phi_pad[CI:MTP, :], in_=zpad_d[:, :])
            wgt_raw = sing.tile([MTP, NMT * C], f32, tag="wgtraw")
            wgt_b16 = sing.tile([MTP, NMT * C], bf16, tag="wgtb16")
            scr_v = sing.tile([MTP, NLOC], bf16, tag="scrv")

            # -------- early sync barrier --------
            # The 8 cores start with multi-us NEFF-launch skew; every
            # AllReduce rendezvous re-exposes it. A dummy 1-element
            # AllReduce first thing absorbs the skew during the DMA-load
            # phase, so the real denominator exchanges run ~6us.
            bar_in = dpool.tile([1, 1], f32, tag="barin")
            bar_out = dpool.tile([1, 1], f32, tag="barout")
            bar_s = sing.tile([1, 1], f32, tag="bars")
            nc.gpsimd.memset(bar_s[:, :], 1.0)
            nc.gpsimd.dma_start(out=bar_in, in_=bar_s)
            nc.gpsimd.collective_compute(
                "AllReduce",
                ALU.add,
                replica_groups=REPLICA_GROUPS,
                ins=[bar_in.opt()],
                outs=[bar_out.opt()],
            )

            # -------- PE warm-up burst --------
            # ~5us of dense full-array (K=128) matmuls so the HAM
            # clock-gate releases (1.2 -> 2.4 GHz): partial-array work
            # (K=32 fT, K=65 wgT) does not register enough activity.
            # Sources a memset scratch (no DMA dependency -> starts right
            # after the preamble, overlapping the input loads) and rotates
            # 4 PSUM banks so WAW never stalls the PE.
            warm_src = sing.tile([MTP, CK], bf16, tag="warmsrc")
            nc.vector.memset(warm_src[:, :], 0.0)
            warm_ctx = ExitStack()
            warmp = warm_ctx.enter_context(
                tc.tile_pool(name="warmp", bufs=4, space="PSUM")
            )
            for k in range(12):
                wps = warmp.tile([MTP, CK], f32, tag="warm", name=f"warm{k}")
                nc.tensor.matmul(
                    wps,
                    lhsT=warm_src[:, 0:MTP],
                    rhs=warm_src[:, :],
                    start=True,
                    stop=True,
                )
            warm_ctx.close()

            # -------- projections --------
            # Bias adds alternate ACT/DVE so neither serializes the phase.
            psA_ctx = ExitStack()
            psA = psA_ctx.enter_context(
                tc.tile_pool(name="psA", bufs=4, space="PSUM")
            )
            proj_alt = [0]

            def emit_add(dst, ps, bias):
                # first chain (phi0/th0/th1 gate the first fT) alternates
                # ACT/DVE; everything later goes to DVE (ACT is the
                # stream bottleneck, DVE has early slack)
                if proj_alt[0] in (0, 2):
                    nc.scalar.activation(out=dst, in_=ps, func=AF.Identity, bias=bias)
                else:
                    nc.vector.tensor_scalar_add(dst, ps, bias)
                proj_alt[0] += 1

            def emit_phi(j):
                ps = psA.tile([CI, CK], f32, tag="projps", name=f"phi_ps{j}")
                nc.tensor.matmul(
                    ps,
                    lhsT=pw,
                    rhs=refa[0:C, j * CK : (j + 1) * CK],
                    start=True,
                    stop=True,
                )
                emit_add(phi_pad[0:CI, j * CK : (j + 1) * CK], ps, pb)

            def emit_theta(j):
                ps = psA.tile([CI, CK], f32, tag="projps", name=f"th_ps{j}")
                nc.tensor.matmul(
                    ps,
                    lhsT=tw,
                    rhs=supp_bf[:, j * CK : (j + 1) * CK],
                    start=True,
                    stop=True,
                )
                emit_add(theta_pad[0:CI, j * CK : (j + 1) * CK], ps, tb)

            # phi chunk 0 + theta first: unblocks fT of tiles 0-3
            emit_phi(0)
            for j in range(NLOC // CK):
                emit_theta(j)
            for j in range(1, N // CK):
                emit_phi(j)
            psA_ctx.close()

            ftp = ctx.enter_context(tc.tile_pool(name="ftp", bufs=3, space="PSUM"))
            wgt_ctx = ExitStack()
            wgtp = wgt_ctx.enter_context(
                tc.tile_pool(name="wgtp", bufs=2, space="PSUM")
            )

            state = {"z": None}
            wgt_queue = list(range(NMT))
            ets = [None] * NMT
            srecs = [None] * NG

            def emit_wgt(mt):
                ps = wgtp.tile([MTP, C], f32, tag="wgtps")
                nc.tensor.matmul(
                    ps,
                    lhsT=refa[:, mt * MTP : (mt + 1) * MTP],
                    rhs=wga,
                    start=True,
                    stop=True,
                )
                nc.vector.tensor_copy(wgt_raw[:, mt * C : (mt + 1) * C], ps)

            def emit_c(mt):
                g = group_of[mt]
                tl = mt - group_start[g]
                nc.vector.tensor_scalar_mul(
                    wgt_b16[:, mt * C : (mt + 1) * C],
                    wgt_raw[:, mt * C : (mt + 1) * C],
                    srecs[g][:, tl : tl + 1],
                )
                # col-tiled z: partitions 0:64 accumulate n 0:1024,
                # partitions 64:128 accumulate n 1024:2048
                z = state["z"]
                w = wgt_b16[:, mt * C : (mt + 1) * C]
                e = ets[mt]
                for jj in range(2):
                    for ph in range(2):
                        nc.tensor.matmul(
                            z[ph * C : (ph + 1) * C, jj * CK : (jj + 1) * CK],
                            lhsT=w,
                            rhs=e[:, ph * 1024 + jj * CK : ph * 1024 + (jj + 1) * CK],
                            start=(mt == 0),
                            stop=(mt == NMT - 1),
                        )

            # Emission-time model (times relative to first exp, ACT-paced).
            TILE_T = 2.1
            CC_GAP = 2.5
            MARGIN = 2.0
            PE_LAG = 2
            cc_land = [None] * NG
            c_ready = []

            def dribble(mt):
                # wgT matmuls 4/slot on slots 2..9; then z work, <=3/slot
                if wgt_queue:
                    if mt >= 2:
                        for _ in range(4):
                            emit_wgt(wgt_queue.pop(0))
                        if not wgt_queue:
                            wgt_ctx.close()
                            zpp = ctx.enter_context(
                                tc.tile_pool(name="zpp", bufs=1, space="PSUM")
                            )
                            state["z"] = zpp.tile(
                                [MTP, NLOC // 2], f32, tag="z", name="z_ps"
                            )
                    return
                pe_now = (mt - PE_LAG) * TILE_T
                budget = 3
                while budget and c_ready:
                    mt2 = c_ready[0]
                    land = cc_land[group_of[mt2]]
                    if land is not None and pe_now >= land + MARGIN:
                        emit_c(c_ready.pop(0))
                        budget -= 1
                    else:
                        break

            pend_dsum = []
            for g, gs in enumerate(GROUP_SIZES):
                s2 = spool.tile([MTP, 2 * gs], f32, tag=f"s2{g}")
                nc.gpsimd.memset(s2[:, :], 0.0)
                d_lag = 0.3
                for tl in range(gs):
                    mt = group_start[g] + tl
                    et = epool.tile([MTP, NLOC], bf16, tag="et", name=f"et{mt}")
                    ets[mt] = et
                    for hh in range(2):
                        ft = ftp.tile([MTP, 2 * CK], f32, tag="ft")
                        for jj in range(2):
                            j = 2 * hh + jj
                            nc.tensor.matmul(
                                ft[:, jj * CK : (jj + 1) * CK],
                                lhsT=phi_pad[:, mt * MTP : (mt + 1) * MTP],
                                rhs=theta_pad[:, j * CK : (j + 1) * CK],
                                start=True,
                                stop=True,
                            )
                        dst = et[:, hh * 2 * CK : (hh + 1) * 2 * CK]
                        if mt in D_TILES:
                            nc.vector.tensor_scalar(
                                out=dst.bitcast(i16),
                                in0=ft[:, :],
                                scalar1=SCH_A,
                                scalar2=SCH_B,
                                op0=ALU.mult,
                                op1=ALU.add,
                            )
                        else:
                            nc.scalar.activation(
                                out=dst,
                                in_=ft[:, :],
                                func=AF.Exp,
                                accum_out=s2[:, 2 * tl + hh : 2 * tl + hh + 1],
                            )
                    # defer the D-tile sum by one tile: keeps DVE free to
                    # evacuate the next tile's ft promptly (ft buffers are
                    # the ACT stream's lifeline)
                    if pend_dsum and (mt not in D_TILES or tl == gs - 1):
                        pmt, pcol = pend_dsum.pop(0)
                        nc.vector.tensor_scalar(
                            out=scr_v[:, :],
                            in0=ets[pmt][:, :],
                            scalar1=1.0,
                            scalar2=0.0,
                            op0=ALU.mult,
                            op1=ALU.add,
                            accum_out=pcol,
                        )
                    if mt in D_TILES:
                        pend_dsum.append((mt, s2[:, 2 * tl : 2 * tl + 1]))
                        d_lag = 3.0
                        if tl == gs - 1:
                            pmt, pcol = pend_dsum.pop(0)
                            nc.vector.tensor_scalar(
                                out=scr_v[:, :],
                                in0=ets[pmt][:, :],
                                scalar1=1.0,
                                scalar2=0.0,
                                op0=ALU.mult,
                                op1=ALU.add,
                                accum_out=pcol,
                            )
                    dribble(mt)
                # group complete: exchange softmax denominators (keep the
                # gpsimd queue empty so the CC trigger fires promptly)
                stot = spool.tile([MTP, gs], f32, tag=f"stot{g}")
                nc.gpsimd.tensor_add(
                    stot,
                    s2[:, :].rearrange("p (t q) -> p q t", q=2)[:, 0, :],
                    s2[:, :].rearrange("p (t q) -> p q t", q=2)[:, 1, :],
                )
                cin = dpool.tile([MTP, gs], f32, tag=f"cin{g}")
                cout = dpool.tile([MTP, gs], f32, tag=f"cout{g}")
                nc.sync.dma_start(out=cin, in_=stot)
                nc.gpsimd.collective_compute(
                    "AllReduce",
                    ALU.add,
                    replica_groups=REPLICA_GROUPS,
                    ins=[cin.opt()],
                    outs=[cout.opt()],
                )
                ssum = spool.tile([MTP, gs], f32, tag=f"ssum{g}")
                nc.sync.dma_start(out=ssum, in_=cout)
                srec = spool.tile([MTP, gs], f32, tag=f"srec{g}")
                nc.vector.reciprocal(out=srec, in_=ssum)
                srecs[g] = srec
                launch = (group_start[g] + gs) * TILE_T + d_lag + 1.2
                cc_dur = 7.0
                cc_land[g] = max(
                    launch + cc_dur,
                    16.0 if g == 0 else cc_land[g - 1] + CC_GAP,
                )
                c_ready.extend(range(group_start[g], group_start[g] + gs))
                if g == 1:
                    # dependency-free mid-stream re-sync: the pair drifts
                    # apart after the initial barrier, inflating the later
                    # (tail-critical) AllReduces from ~6 to ~10us
                    bar2_in = dpool.tile([1, 1], f32, tag="bar2in")
                    bar2_out = dpool.tile([1, 1], f32, tag="bar2out")
                    nc.gpsimd.dma_start(out=bar2_in, in_=bar_s)
                    nc.gpsimd.collective_compute(
                        "AllReduce",
                        ALU.add,
                        replica_groups=REPLICA_GROUPS,
                        ins=[bar2_in.opt()],
                        outs=[bar2_out.opt()],
                    )

            while c_ready:
                emit_c(c_ready.pop(0))

            # ---------------- epilogue ----------------
            for jj in range(2):
                e2 = outp.tile([MTP, CK], f32, tag="e2")
                nc.vector.scalar_tensor_tensor(
                    out=e2,
                    in0=state["z"][:, jj * CK : (jj + 1) * CK],
                    scalar=wb[:, :],
                    in1=supp_t[:, jj * CK : (jj + 1) * CK],
                    op0=ALU.add,
                    op1=ALU.add,
                )
                nc.sync.dma_start(
                    out=out[:, jj * CK : (jj + 1) * CK], in_=e2
                )

    nc.compile()
    return nc


def _get_nc():
    if "nc" not in _cache:
        _cache["nc"] = _build()
    return _cache["nc"]


def kernel(
    supp_feature,
    ref_feature,
    theta_w,
    theta_b,
    phi_w,
    phi_b,
    g_w,
    g_b,
    w_w,
    w_b,
    _trace=False,
):
    import ml_dtypes

    # run_bass_kernel_spmd imports antenv.axon_hooks when tracing is
    # requested; this container's antenv stub lacks that module, so
    # provide a no-op fallback.
    try:
        import antenv.axon_hooks  # noqa: F401
    except ImportError:
        import sys
        import types

        import antenv

        _mod = types.ModuleType("antenv.axon_hooks")
        _mod._hook = None
        _mod.get_axon_ntff_profile_hook = lambda: _mod._hook
        _mod.set_axon_ntff_profile_hook = lambda h: setattr(_mod, "_hook", h)
        sys.modules["antenv.axon_hooks"] = _mod
        antenv.axon_hooks = _mod

    from concourse.bass_utils import run_bass_kernel_spmd

    bf = ml_dtypes.bfloat16
    supp_feature = np.asarray(supp_feature, dtype=np.float32)
    ref_feature = np.asarray(ref_feature, dtype=np.float32)
    theta_w = np.asarray(theta_w, dtype=np.float32)
    theta_b = np.asarray(theta_b, dtype=np.float32)
    phi_w = np.asarray(phi_w, dtype=np.float32)
    phi_b = np.asarray(phi_b, dtype=np.float32)
    g_w = np.asarray(g_w, dtype=np.float32)
    g_b = np.asarray(g_b, dtype=np.float32)
    w_w = np.asarray(w_w, dtype=np.float32)
    w_b = np.asarray(w_b, dtype=np.float32)

    nc = _get_nc()

    supp2 = supp_feature.reshape(B, C, N)
    ref2 = ref_feature.reshape(B, C, N)
    # Fold the output 1x1 conv into g (weight-only transform):
    #   w_w @ (g_w @ ref + g_b) = (w_w@g_w) @ ref + (w_w@g_b)
    Wg = (w_w @ g_w).astype(np.float32)
    wgb = (w_w @ g_b).astype(np.float32)
    wg_aug = np.concatenate([Wg.T, wgb[None, :]], axis=0)  # [65, 64]
    wcomb = np.zeros((MTP, 160), np.float32)
    wcomb[0:C, 0:CI] = theta_w.T
    wcomb[0:C, CI : 2 * CI] = phi_w.T
    wcomb[0 : C + 1, C + CI : C + CI + C] = wg_aug
    wcomb = np.ascontiguousarray(wcomb.astype(bf))
    tbpb = np.ascontiguousarray(
        np.stack(
            [np.concatenate([theta_b, theta_b]), np.concatenate([phi_b, phi_b])],
            axis=1,
        ).astype(np.float32)
    )
    w_bc2 = np.ascontiguousarray(
        np.concatenate([w_b, w_b]).reshape(MTP, 1).astype(np.float32)
    )
    zpad = np.zeros((MTP - CI, N), bf)

    in_maps = []
    for core in range(NCORES):
        b, h = core // 2, core % 2
        ref_aug = np.ascontiguousarray(
            np.concatenate(
                [ref2[b], np.ones((1, N), np.float32)], axis=0
            ).astype(bf)
        )
        sloc = supp2[b, :, h * NLOC : (h + 1) * NLOC]
        supp_rp = np.ascontiguousarray(
            sloc.reshape(C, 2, NLOC // 2).transpose(1, 0, 2).reshape(MTP, NLOC // 2)
        )
        in_maps.append(
            {
                "supp_rp": supp_rp,
                "supp_b": np.ascontiguousarray(sloc.astype(bf)),
                "ref_aug": ref_aug,
                "wcomb": wcomb,
                "tbpb": tbpb,
                "w_bc2": w_bc2,
                "zpad": zpad,
            }
        )

    res = run_bass_kernel_spmd(
        nc, in_maps, list(range(NCORES)), trace=_trace
    )
    if _trace:
        _cache["last_exec_time_ns"] = res.exec_time_ns
        _cache["last_results"] = res

    z = np.empty((B, C, N), dtype=np.float32)
    for core in range(NCORES):
        b, h = core // 2, core % 2
        o = res.results[core]["out"]  # [128, 1024]
        z[b, :, h * NLOC : h * NLOC + NLOC // 2] = o[0:C]
        z[b, :, h * NLOC + NLOC // 2 : (h + 1) * NLOC] = o[C:MTP]
    return z.reshape(B, C, H, W)
